# revision 4
# baseline (speedup 1.0000x reference)
"""Trainium2 Bass kernel for the memristor-crossbar layer (nn_CustomLayer_30588757082254).

out = unmap(x @ G_eff) + bias, where G_eff = 1/(1/G + R_par) is an elementwise
transform of weight.T with globally min/max-normalized conductances.

Strategy: data-parallel over batch (8 cores x 1024 rows), single-phase N-split
schedule (no DRAM staging of partials).

Math (S = 1/s folds the output unmapping scale into the transform; kappa is
folded into the transformed weights so the row-sum correction disappears):
  s = (g_max-g_min)/(wmax-wmin);  a = g_min/s - wmin;  kappa = -a
  u  = WT + a            (ACT, per-partition bias)
  w  = 1/u + c           (DVE fast recip in-place, then GpSimd add of the
                          host-precomputed full parasitic matrix c = s*R_par)
  ge = 1/w + kappa       (DVE fast recip in-place, ACT bias add -> f32r)
  out = x @ ge + bias    (PE; bias added into PSUM via ones-row matmul trick)

Schedule per core: transform streams 32 half-tiles [128,1024] (2 col-halves x
16 k-tiles) through ACT -> DVE -> GpSimd -> DVE -> ACT; matmuls consume ge in
4 column-quarters of 512 (one PSUM bank per (quarter, m-block), all 8 banks
live), kt-outer / mb-inner, with the bias matmul closing each group and an ACT
copy draining PSUM to SBUF for the output DMA. Emission order
[T h0][q0][T h1][q1][q2][q3] lets the h1 transform overlap the q0/q1 sweeps.
Host prep is layout + scalar weight stats (wmin/wmax -> s, a, kappa) + the
index-derived parasitic matrix, as in the two-phase baseline.
"""
import numpy as np

import concourse.bass as bass
import concourse.mybir as mybir
import concourse.tile as tile
from concourse import bacc
from concourse.bass_utils import run_bass_kernel_spmd
from concourse.dve_ops import RECIP_APPROX_FAST_CONSTS, RECIPROCAL_APPROX_FAST

F32 = mybir.dt.float32
F32R = mybir.dt.float32r
AF = mybir.ActivationFunctionType
ALU = mybir.AluOpType
CRC = RECIP_APPROX_FAST_CONSTS

N_CORES = 8
B, K, N = 8192, 2048, 2048
BC = B // N_CORES            # 1024 batch rows per core
KT = K // 128                # 16 k-tiles
MB = BC // 128               # 8 m-blocks per core
NH = 2                       # transform col-halves of 1024
NQ = 4                       # psum col-quarters of 512

PARASITIC_R = 2.0
G_MIN, G_MAX = 1.0 / 100000.0, 1.0 / 1000.0

_CACHE = {}


def _build_nc():
    nc = bacc.Bacc("TRN2", target_bir_lowering=False, debug=False,
                   num_devices=N_CORES)
    # wt/cf blocked as [h, kt] -> [128, 1024] tiles, rows contiguous per tile
    wt_in = nc.dram_tensor("wt", [NH * KT * 128, 1024], F32,
                           kind="ExternalInput")
    cf_in = nc.dram_tensor("cf", [NH * KT * 128, 1024], F32,
                           kind="ExternalInput")
    xt_in = nc.dram_tensor("xt", [128, KT * BC], F32R, kind="ExternalInput")
    bias_in = nc.dram_tensor("bias", [1, N], F32R, kind="ExternalInput")
    mmx_in = nc.dram_tensor("mmx", [128, 4], F32, kind="ExternalInput")
    out_d = nc.dram_tensor("out", [BC, N], F32, kind="ExternalOutput")

    with tile.TileContext(nc) as tc:
        with (
            tc.tile_pool(name="wtp", bufs=4) as wtp,
            tc.tile_pool(name="cfp", bufs=4) as cfp,
            tc.tile_pool(name="gep", bufs=20) as gep,
            tc.tile_pool(name="xtp", bufs=KT) as xtp,
            tc.tile_pool(name="osbp", bufs=4) as osbp,
            tc.tile_pool(name="smallp", bufs=1) as sp,
            tc.tile_pool(name="pcp", bufs=8, space="PSUM") as pcp,
        ):
            # ---------------- tiny inputs ----------------
            with nc.named_scope("setup"):
                bcv = sp.tile([128, 4], F32, tag="bcv")
                nc.sync.dma_start(out=bcv[:], in_=mmx_in[:])
                bias_row = sp.tile([1, N], F32R, tag="bias_row")
                nc.sync.dma_start(out=bias_row[:], in_=bias_in[:])
                ones_row_f = sp.tile([1, 128], F32, tag="ones_row_f")
                nc.vector.memset(ones_row_f[:], 1.0)
                ones_row = sp.tile([1, 128], F32R, tag="ones_row")
                nc.vector.tensor_copy(ones_row[:], ones_row_f[:])
                # trigger the lazy ACT table load before real work arrives
                warm = sp.tile([1, 2], F32, tag="warm")
                nc.vector.memset(warm[:], 0.0)
                nc.scalar.activation(warm[:], warm[:], AF.Identity,
                                     bias=0.0, scale=1.0)
            a_b = bcv[:, 0:1]
            kap_b = bcv[:, 1:2]

            xt_t = {}
            ge = {}

            def transform_half(h):
                """ge[h, kt] = 1/(1/(WT+a) + c) + kappa, tiles [128, 1024]."""
                for kt in range(KT):
                    r0 = (h * KT + kt) * 128
                    w_t = wtp.tile([128, 1024], F32, tag="wt",
                                   name=f"wt{h}_{kt}")
                    nc.sync.dma_start(out=w_t[:], in_=wt_in[r0:r0 + 128, :])
                    c_t = cfp.tile([128, 1024], F32, tag="cf",
                                   name=f"cf{h}_{kt}")
                    nc.scalar.dma_start(out=c_t[:], in_=cf_in[r0:r0 + 128, :])
                    if h == 0:
                        # interleave x slices so they trail wt/cf per k-tile
                        x_t = xtp.tile([128, BC], F32R, tag="xt",
                                       name=f"xt{kt}")
                        nc.gpsimd.dma_start(
                            out=x_t[:], in_=xt_in[:, kt * BC:(kt + 1) * BC])
                        xt_t[kt] = x_t
                    # u = WT + a (in-place on wt tile)
                    nc.scalar.activation(w_t[:], w_t[:], AF.Identity,
                                         bias=a_b, scale=1.0)
                    # 1/u (in-place)
                    nc.vector._custom_dve(RECIPROCAL_APPROX_FAST, out=w_t[:],
                                          in0=w_t[:], s0=CRC["s0"],
                                          s1=CRC["s1"], imm2=CRC["imm2"])
                    # w = 1/u + c (in-place on cf tile)
                    nc.gpsimd.tensor_tensor(c_t[:], c_t[:], w_t[:], ALU.add)
                    # 1/w (in-place)
                    nc.vector._custom_dve(RECIPROCAL_APPROX_FAST, out=c_t[:],
                                          in0=c_t[:], s0=CRC["s0"],
                                          s1=CRC["s1"], imm2=CRC["imm2"])
                    # ge = 1/w + kappa -> f32r
                    g_t = gep.tile([128, 1024], F32R, tag="ge",
                                   name=f"ge{h}_{kt}")
                    nc.scalar.activation(g_t[:], c_t[:], AF.Identity,
                                         bias=kap_b, scale=1.0)
                    ge[h, kt] = g_t

            def quarter_sweep(q):
                h, qq = q // 2, q % 2
                cs = qq * 512
                pcs = {}
                for kt in range(KT):
                    for mb in range(MB):
                        if kt == 0:
                            pcs[mb] = pcp.tile([128, 512], F32, tag="pc",
                                               name=f"p{q}_{mb}")
                        nc.tensor.matmul(
                            pcs[mb][:],
                            xt_t[kt][:, mb * 128:(mb + 1) * 128],
                            ge[h, kt][:, cs:cs + 512],
                            start=(kt == 0), stop=False)
                for mb in range(MB):
                    nc.tensor.matmul(pcs[mb][:], ones_row[:],
                                     bias_row[:, q * 512:(q + 1) * 512],
                                     start=False, stop=True)
                    osb = osbp.tile([128, 512], F32, tag="osb",
                                    name=f"o{q}_{mb}")
                    nc.scalar.copy(osb[:], pcs[mb][:])
                    nc.scalar.dma_start(
                        out=out_d[mb * 128:(mb + 1) * 128,
                                  q * 512:(q + 1) * 512],
                        in_=osb[:])

            with nc.named_scope("t_h0"):
                transform_half(0)
            with nc.named_scope("q0"):
                quarter_sweep(0)
            with nc.named_scope("t_h1"):
                transform_half(1)
            with nc.named_scope("q1"):
                quarter_sweep(1)
            with nc.named_scope("q2"):
                quarter_sweep(2)
            with nc.named_scope("q3"):
                quarter_sweep(3)
    nc.finalize()
    return nc


def _prep_inputs(x, weight, bias):
    wtT = np.ascontiguousarray(weight.T)          # [K, N]
    wmin = float(wtT.min())
    wmax = float(wtT.max())
    s = (G_MAX - G_MIN) / (wmax - wmin)
    a = G_MIN / s - wmin
    kappa = wmin - G_MIN / s
    mmx1 = np.zeros((1, 4), dtype=np.float32)
    mmx1[0, 0] = a
    mmx1[0, 1] = kappa
    mmx = np.ascontiguousarray(np.broadcast_to(mmx1, (128, 4)))

    # wt blocked [h, kt] -> [128, 1024]
    wt_b = np.ascontiguousarray(
        wtT.reshape(KT, 128, NH, 1024).transpose(2, 0, 1, 3)
        .reshape(NH * KT * 128, 1024))
    # full parasitic matrix c = s * R_par = s*(4098 + 2n - 2*row), same layout
    row = np.arange(K, dtype=np.float64)[:, None]
    coln = np.arange(N, dtype=np.float64)[None, :]
    cf = (np.float64(s) * (4098.0 + 2.0 * coln - 2.0 * row)).astype(np.float32)
    cf_b = np.ascontiguousarray(
        cf.reshape(KT, 128, NH, 1024).transpose(2, 0, 1, 3)
        .reshape(NH * KT * 128, 1024))

    bias2 = np.ascontiguousarray(bias.reshape(1, N)).astype(np.float32)
    in_maps = []
    for c in range(N_CORES):
        x_c = x[c * BC:(c + 1) * BC, :]           # [BC, K]
        # xh[p, kt, m] = x_c[m, kt*128+p]
        xh = np.ascontiguousarray(
            x_c.T.reshape(KT, 128, BC).transpose(1, 0, 2).reshape(128, KT * BC))
        in_maps.append({"wt": wt_b, "cf": cf_b, "xt": xh, "bias": bias2,
                        "mmx": mmx})
    return in_maps


def _run(x, weight, bias, trace=False, trace_kwargs=None):
    if "nc" not in _CACHE:
        _CACHE["nc"] = _build_nc()
    nc = _CACHE["nc"]
    in_maps = _prep_inputs(x, weight, bias)
    res = run_bass_kernel_spmd(nc, in_maps, list(range(N_CORES)), trace=trace,
                               **(trace_kwargs or {}))
    out = np.concatenate([res.results[c]["out"] for c in range(N_CORES)], axis=0)
    return out, res


def kernel(x, weight, bias):
    x = np.asarray(x, dtype=np.float32)
    weight = np.asarray(weight, dtype=np.float32)
    bias = np.asarray(bias, dtype=np.float32)
    out, _ = _run(x, weight, bias, trace=False)
    return out.astype(np.float32)


# revision 5
# speedup vs baseline: 1.0498x; 1.0498x over previous
"""Trainium2 Bass kernel for the memristor-crossbar layer (nn_CustomLayer_30588757082254).

out = unmap(x @ G_eff) + bias, where G_eff = 1/(1/G + R_par) is an elementwise
transform of weight.T with globally min/max-normalized conductances.

Strategy: data-parallel over batch (8 cores x 1024 rows), single-phase N-split
schedule (no DRAM staging of partials, no streamed parasitic matrix).

Math (S = 1/s folds the unmapping scale into the transform; kappa is folded
into the transformed weights so the row-sum correction disappears; the
parasitic term c = s*R_par is separable, c = c0b[n] - rp[row], so it is
applied from a persistent column tile + per-k-tile partition vector instead
of a streamed matrix):
  s = (g_max-g_min)/(wmax-wmin);  a = g_min/s - wmin;  kappa = -a
  u  = WT + a                  (ACT, per-partition bias; WT streamed as fp16)
  z  = (c0b - rp_kt) * u       (DVE scalar_tensor_tensor)
  v  = z + 1                   (ACT, ones bias, in-place)
  q  = kappa*v + u             (DVE affine_then_add)
  r  = 1/v                     (DVE fast recip, in-place)
  ge = q * r                   (GpSimd mult -> f32r)   [ge = S*G_eff + kappa]
  out = x @ ge + bias          (PE; bias added into PSUM via ones-row matmul)

Schedule per core: transform streams 32 half-tiles [128,1024] (2 col-halves x
16 k-tiles) through ACT/DVE/GpSimd; matmuls consume ge in 4 column-quarters
of 512 (one PSUM bank per (quarter, m-block), all 8 banks live), kt-outer /
mb-inner, bias matmul closes each group, ACT copy drains PSUM for the output
DMA. Emission order [T h0 + x stream][q0][T h1][q1][q2][q3]. Host prep is
layout + dtype cast + scalar weight stats + index-derived parasitic vectors.
"""
import numpy as np

import concourse.bass as bass
import concourse.mybir as mybir
import concourse.tile as tile
from concourse import bacc
from concourse.bass_utils import run_bass_kernel_spmd
from concourse.dve_ops import RECIP_APPROX_FAST_CONSTS, RECIPROCAL_APPROX_FAST

F32 = mybir.dt.float32
F32R = mybir.dt.float32r
F16 = mybir.dt.float16
AF = mybir.ActivationFunctionType
ALU = mybir.AluOpType
CRC = RECIP_APPROX_FAST_CONSTS

N_CORES = 8
B, K, N = 8192, 2048, 2048
BC = B // N_CORES            # 1024 batch rows per core
KT = K // 128                # 16 k-tiles
MB = BC // 128               # 8 m-blocks per core
NH = 2                       # transform col-halves of 1024
NQ = 4                       # psum col-quarters of 512

PARASITIC_R = 2.0
G_MIN, G_MAX = 1.0 / 100000.0, 1.0 / 1000.0

_CACHE = {}


def _build_nc():
    nc = bacc.Bacc("TRN2", target_bir_lowering=False, debug=False,
                   num_devices=N_CORES)
    # wt blocked as [h, kt] -> [128, 1024] tiles, rows contiguous per tile
    wt_in = nc.dram_tensor("wt", [NH * KT * 128, 1024], F16,
                           kind="ExternalInput")
    xt_in = nc.dram_tensor("xt", [128, KT * BC], F32R, kind="ExternalInput")
    bias_in = nc.dram_tensor("bias", [1, N], F32R, kind="ExternalInput")
    mmx_in = nc.dram_tensor("mmx", [128, 24], F32, kind="ExternalInput")
    c0_in = nc.dram_tensor("c0", [128, N], F32, kind="ExternalInput")
    out_d = nc.dram_tensor("out", [BC, N], F32, kind="ExternalOutput")

    with tile.TileContext(nc) as tc:
        with (
            tc.tile_pool(name="wtp", bufs=4) as wtp,
            tc.tile_pool(name="up", bufs=4) as up,
            tc.tile_pool(name="zp", bufs=4) as zp,
            tc.tile_pool(name="gep", bufs=18) as gep,
            tc.tile_pool(name="xtp", bufs=KT) as xtp,
            tc.tile_pool(name="osbp", bufs=4) as osbp,
            tc.tile_pool(name="smallp", bufs=1) as sp,
            tc.tile_pool(name="pcp", bufs=8, space="PSUM") as pcp,
        ):
            # ---------------- tiny inputs ----------------
            with nc.named_scope("setup"):
                bcv = sp.tile([128, 24], F32, tag="bcv")
                nc.sync.dma_start(out=bcv[:], in_=mmx_in[:])
                c0b = sp.tile([128, N], F32, tag="c0b")
                nc.sync.dma_start(out=c0b[:], in_=c0_in[:])
                bias_row = sp.tile([1, N], F32R, tag="bias_row")
                nc.sync.dma_start(out=bias_row[:], in_=bias_in[:])
                ones_row_f = sp.tile([1, 128], F32, tag="ones_row_f")
                nc.vector.memset(ones_row_f[:], 1.0)
                ones_row = sp.tile([1, 128], F32R, tag="ones_row")
                nc.vector.tensor_copy(ones_row[:], ones_row_f[:])
                # trigger the lazy ACT table load before real work arrives
                warm = sp.tile([1, 2], F32, tag="warm")
                nc.vector.memset(warm[:], 0.0)
                nc.scalar.activation(warm[:], warm[:], AF.Identity,
                                     bias=0.0, scale=1.0)
            a_b = bcv[:, 0:1]
            kap_b = bcv[:, 1:2]
            one_b = bcv[:, 2:3]
            rpn = bcv[:, 8:24]            # -2*s*(128*kt + p), col kt

            xt_t = {}
            ge = {}

            def transform_half(h):
                """ge[h, kt] = S*G_eff + kappa, tiles [128, 1024]."""
                for kt in range(KT):
                    r0 = (h * KT + kt) * 128
                    w_t = wtp.tile([128, 1024], F16, tag="wt",
                                   name=f"wt{h}_{kt}")
                    nc.sync.dma_start(out=w_t[:], in_=wt_in[r0:r0 + 128, :])
                    if h == 0:
                        # interleave x slices so they trail wt per k-tile
                        x_t = xtp.tile([128, BC], F32R, tag="xt",
                                       name=f"xt{kt}")
                        nc.gpsimd.dma_start(
                            out=x_t[:], in_=xt_in[:, kt * BC:(kt + 1) * BC])
                        xt_t[kt] = x_t
                    # u = WT + a
                    u_t = up.tile([128, 1024], F32, tag="u", name=f"u{h}_{kt}")
                    nc.scalar.activation(u_t[:], w_t[:], AF.Identity,
                                         bias=a_b, scale=1.0)
                    # z = (c0b - rp_kt) * u
                    z_t = zp.tile([128, 1024], F32, tag="z", name=f"z{h}_{kt}")
                    nc.vector.scalar_tensor_tensor(
                        z_t[:], c0b[:, h * 1024:(h + 1) * 1024],
                        rpn[:, kt:kt + 1], u_t[:], ALU.add, ALU.mult)
                    # v = z + 1 (in-place)
                    nc.scalar.activation(z_t[:], z_t[:], AF.Identity,
                                         bias=one_b, scale=1.0)
                    # q = kappa*v + u (in-place on u tile)
                    nc.vector.affine_then_add(u_t[:], z_t[:], u_t[:],
                                              kap_b, 0.0)
                    # r = 1/v (in-place)
                    nc.vector._custom_dve(RECIPROCAL_APPROX_FAST, out=z_t[:],
                                          in0=z_t[:], s0=CRC["s0"],
                                          s1=CRC["s1"], imm2=CRC["imm2"])
                    # ge = q * r
                    g_t = gep.tile([128, 1024], F32R, tag="ge",
                                   name=f"ge{h}_{kt}")
                    nc.gpsimd.tensor_tensor(g_t[:], u_t[:], z_t[:], ALU.mult)
                    ge[h, kt] = g_t

            def quarter_sweep(q):
                h, qq = q // 2, q % 2
                cs = qq * 512
                pcs = {}
                for kt in range(KT):
                    for mb in range(MB):
                        if kt == 0:
                            pcs[mb] = pcp.tile([128, 512], F32, tag="pc",
                                               name=f"p{q}_{mb}")
                        nc.tensor.matmul(
                            pcs[mb][:],
                            xt_t[kt][:, mb * 128:(mb + 1) * 128],
                            ge[h, kt][:, cs:cs + 512],
                            start=(kt == 0), stop=False)
                for mb in range(MB):
                    nc.tensor.matmul(pcs[mb][:], ones_row[:],
                                     bias_row[:, q * 512:(q + 1) * 512],
                                     start=False, stop=True)
                    osb = osbp.tile([128, 512], F32, tag="osb",
                                    name=f"o{q}_{mb}")
                    nc.scalar.copy(osb[:], pcs[mb][:])
                    nc.scalar.dma_start(
                        out=out_d[mb * 128:(mb + 1) * 128,
                                  q * 512:(q + 1) * 512],
                        in_=osb[:])

            with nc.named_scope("t_h0"):
                transform_half(0)
            with nc.named_scope("q0"):
                quarter_sweep(0)
            with nc.named_scope("t_h1"):
                transform_half(1)
            with nc.named_scope("q1"):
                quarter_sweep(1)
            with nc.named_scope("q2"):
                quarter_sweep(2)
            with nc.named_scope("q3"):
                quarter_sweep(3)
    nc.finalize()
    return nc


def _prep_inputs(x, weight, bias):
    wtT = np.ascontiguousarray(weight.T)          # [K, N]
    wmin = float(wtT.min())
    wmax = float(wtT.max())
    s = (G_MAX - G_MIN) / (wmax - wmin)
    a = G_MIN / s - wmin
    kappa = wmin - G_MIN / s
    p_idx = np.arange(128, dtype=np.float64)
    mmx = np.zeros((128, 24), dtype=np.float32)
    mmx[:, 0] = a
    mmx[:, 1] = kappa
    mmx[:, 2] = 1.0
    for kt in range(KT):
        mmx[:, 8 + kt] = (-2.0 * s * (128.0 * kt + p_idx)).astype(np.float32)
    # c0b[p, n] = s*(4098 + 2n), same for all partitions
    coln = np.arange(N, dtype=np.float64)[None, :]
    c0 = np.ascontiguousarray(np.broadcast_to(
        (np.float64(s) * (4098.0 + 2.0 * coln)).astype(np.float32), (128, N)))

    # wt blocked [h, kt] -> [128, 1024], cast to fp16
    wt_b = np.ascontiguousarray(
        wtT.reshape(KT, 128, NH, 1024).transpose(2, 0, 1, 3)
        .reshape(NH * KT * 128, 1024).astype(np.float16))

    bias2 = np.ascontiguousarray(bias.reshape(1, N)).astype(np.float32)
    in_maps = []
    for c in range(N_CORES):
        x_c = x[c * BC:(c + 1) * BC, :]           # [BC, K]
        # xh[p, kt, m] = x_c[m, kt*128+p]
        xh = np.ascontiguousarray(
            x_c.T.reshape(KT, 128, BC).transpose(1, 0, 2).reshape(128, KT * BC))
        in_maps.append({"wt": wt_b, "xt": xh, "bias": bias2, "mmx": mmx,
                        "c0": c0})
    return in_maps


def _run(x, weight, bias, trace=False, trace_kwargs=None):
    if "nc" not in _CACHE:
        _CACHE["nc"] = _build_nc()
    nc = _CACHE["nc"]
    in_maps = _prep_inputs(x, weight, bias)
    res = run_bass_kernel_spmd(nc, in_maps, list(range(N_CORES)), trace=trace,
                               **(trace_kwargs or {}))
    out = np.concatenate([res.results[c]["out"] for c in range(N_CORES)], axis=0)
    return out, res


def kernel(x, weight, bias):
    x = np.asarray(x, dtype=np.float32)
    weight = np.asarray(weight, dtype=np.float32)
    bias = np.asarray(bias, dtype=np.float32)
    out, _ = _run(x, weight, bias, trace=False)
    return out.astype(np.float32)


# revision 6
# speedup vs baseline: 1.4291x; 1.3613x over previous
"""Trainium2 Bass kernel for the memristor-crossbar layer (nn_CustomLayer_30588757082254).

out = unmap(x @ G_eff) + bias, where G_eff = 1/(1/G + R_par) is an elementwise
transform of weight.T with globally min/max-normalized conductances.

Strategy: data-parallel over batch (8 cores x 1024 rows), single-phase N-split
schedule, fp16 streaming throughout (weights, x, transformed weights). fp16
halves SBUF traffic on every pass and on the PE's moving-operand stream --
SBUF port contention between the PE and DVE/ACT was the measured bottleneck
of the f32 variants -- while matmul cost stays 1 cycle/row and PSUM
accumulation stays fp32. All transform values lie in [-10.5, 10.5], far
inside fp16 range, and the 2^-11 rounding is ~20x under the error gate.

Math (S = 1/s folds the unmapping scale into the transform; kappa folded into
the transformed weights kills the row-sum correction; the parasitic term
c = s*R_par is separable, c = c0b[col] - rp[row], applied from a persistent
column tile + per-k-tile partition vector -- nothing streamed):
  s = (g_max-g_min)/(wmax-wmin);  a = g_min/s - wmin;  kappa = -a
  u  = WT + a                 (ACT, per-partition bias; WT streamed fp16)
  iu = 1/u                    (DVE fast recip, in-place)
  w  = (c0b - rp_kt) + iu     (DVE scalar_tensor_tensor, in-place)
  g0 = 1/w                    (DVE fast recip, in-place)
  ge = g0 + kappa             (ACT bias add -> fp16)   [ge = S*G_eff + kappa]
  out = x @ ge + bias         (PE; bias added into PSUM via ones-row matmul)

Schedule per core: transform streams 32 half-tiles [128,1024] (2 col-halves x
16 k-tiles); all ge tiles stay resident so matmul pacing never recycles them.
Matmuls consume ge in 4 column-quarters of 512 (one PSUM bank per (quarter,
m-block), 8 banks live), kt-outer / mb-inner, bias matmul closes each group,
ACT copy drains PSUM for the output DMA. Emission order
[T h0 + x stream][q0][T h1][q1][q2][q3]. Host prep is layout + fp16 cast +
scalar weight stats + index-derived parasitic vectors.
"""
import numpy as np

import concourse.bass as bass
import concourse.mybir as mybir
import concourse.tile as tile
from concourse import bacc
from concourse.bass_utils import run_bass_kernel_spmd
from concourse.dve_ops import RECIP_APPROX_FAST_CONSTS, RECIPROCAL_APPROX_FAST

F32 = mybir.dt.float32
F32R = mybir.dt.float32r
F16 = mybir.dt.float16
AF = mybir.ActivationFunctionType
ALU = mybir.AluOpType
CRC = RECIP_APPROX_FAST_CONSTS

N_CORES = 8
B, K, N = 8192, 2048, 2048
BC = B // N_CORES            # 1024 batch rows per core
KT = K // 128                # 16 k-tiles
MB = BC // 128               # 8 m-blocks per core
NH = 2                       # transform col-halves of 1024

PARASITIC_R = 2.0
G_MIN, G_MAX = 1.0 / 100000.0, 1.0 / 1000.0

_CACHE = {}


def _build_nc():
    nc = bacc.Bacc("TRN2", target_bir_lowering=False, debug=False,
                   num_devices=N_CORES)
    # wt blocked as [h, kt] -> [128, 1024] tiles, rows contiguous per tile
    wt_in = nc.dram_tensor("wt", [NH * KT * 128, 1024], F16,
                           kind="ExternalInput")
    xt_in = nc.dram_tensor("xt", [128, KT * BC], F16, kind="ExternalInput")
    bias_in = nc.dram_tensor("bias", [1, N], F16, kind="ExternalInput")
    mmx_in = nc.dram_tensor("mmx", [128, 4], F32, kind="ExternalInput")
    rp_in = nc.dram_tensor("rp", [128, KT], F16, kind="ExternalInput")
    c0_in = nc.dram_tensor("c0", [128, N], F16, kind="ExternalInput")
    out_d = nc.dram_tensor("out", [BC, N], F32, kind="ExternalOutput")

    with tile.TileContext(nc) as tc:
        with (
            tc.tile_pool(name="wtp", bufs=4) as wtp,
            tc.tile_pool(name="up", bufs=6) as up,
            tc.tile_pool(name="gep", bufs=NH * KT + 2) as gep,
            tc.tile_pool(name="xtp", bufs=KT) as xtp,
            tc.tile_pool(name="osbp", bufs=4) as osbp,
            tc.tile_pool(name="smallp", bufs=1) as sp,
            tc.tile_pool(name="pcp", bufs=8, space="PSUM") as pcp,
        ):
            # ---------------- tiny inputs ----------------
            with nc.named_scope("setup"):
                bcv = sp.tile([128, 4], F32, tag="bcv")
                nc.sync.dma_start(out=bcv[:], in_=mmx_in[:])
                rpn = sp.tile([128, KT], F16, tag="rpn")
                nc.sync.dma_start(out=rpn[:], in_=rp_in[:])
                c0b = sp.tile([128, N], F16, tag="c0b")
                nc.sync.dma_start(out=c0b[:], in_=c0_in[:])
                bias_row = sp.tile([1, N], F16, tag="bias_row")
                nc.sync.dma_start(out=bias_row[:], in_=bias_in[:])
                ones_row_f = sp.tile([1, 128], F32, tag="ones_row_f")
                nc.vector.memset(ones_row_f[:], 1.0)
                ones_row = sp.tile([1, 128], F16, tag="ones_row")
                nc.vector.tensor_copy(ones_row[:], ones_row_f[:])
                # trigger the lazy ACT table load before real work arrives
                warm = sp.tile([1, 2], F32, tag="warm")
                nc.vector.memset(warm[:], 0.0)
                nc.scalar.activation(warm[:], warm[:], AF.Identity,
                                     bias=0.0, scale=1.0)
            a_b = bcv[:, 0:1]
            kap_b = bcv[:, 1:2]

            xt_t = {}
            ge = {}

            def transform_half(h):
                """ge[h, kt] = S*G_eff + kappa, fp16 tiles [128, 1024]."""
                for kt in range(KT):
                    r0 = (h * KT + kt) * 128
                    w_t = wtp.tile([128, 1024], F16, tag="wt",
                                   name=f"wt{h}_{kt}")
                    nc.sync.dma_start(out=w_t[:], in_=wt_in[r0:r0 + 128, :])
                    if h == 0:
                        # interleave x slices so they trail wt per k-tile
                        x_t = xtp.tile([128, BC], F16, tag="xt",
                                       name=f"xt{kt}")
                        nc.gpsimd.dma_start(
                            out=x_t[:], in_=xt_in[:, kt * BC:(kt + 1) * BC])
                        xt_t[kt] = x_t
                    # u = WT + a
                    u_t = up.tile([128, 1024], F16, tag="u", name=f"u{h}_{kt}")
                    nc.scalar.activation(u_t[:], w_t[:], AF.Identity,
                                         bias=a_b, scale=1.0)
                    # iu = 1/u (in-place)
                    nc.vector._custom_dve(RECIPROCAL_APPROX_FAST, out=u_t[:],
                                          in0=u_t[:], s0=CRC["s0"],
                                          s1=CRC["s1"], imm2=CRC["imm2"])
                    # w = (c0b - rp_kt) + iu (in-place)
                    nc.vector.scalar_tensor_tensor(
                        u_t[:], c0b[:, h * 1024:(h + 1) * 1024],
                        rpn[:, kt:kt + 1], u_t[:], ALU.add, ALU.add)
                    # g0 = 1/w (in-place)
                    nc.vector._custom_dve(RECIPROCAL_APPROX_FAST, out=u_t[:],
                                          in0=u_t[:], s0=CRC["s0"],
                                          s1=CRC["s1"], imm2=CRC["imm2"])
                    # ge = g0 + kappa -> fp16
                    g_t = gep.tile([128, 1024], F16, tag="ge",
                                   name=f"ge{h}_{kt}")
                    nc.scalar.activation(g_t[:], u_t[:], AF.Identity,
                                         bias=kap_b, scale=1.0)
                    ge[h, kt] = g_t

            def quarter_sweep(q):
                h, qq = q // 2, q % 2
                cs = qq * 512
                pcs = {}
                for kt in range(KT):
                    for mb in range(MB):
                        if kt == 0:
                            pcs[mb] = pcp.tile([128, 512], F32, tag="pc",
                                               name=f"p{q}_{mb}")
                        nc.tensor.matmul(
                            pcs[mb][:],
                            xt_t[kt][:, mb * 128:(mb + 1) * 128],
                            ge[h, kt][:, cs:cs + 512],
                            start=(kt == 0), stop=False)
                for mb in range(MB):
                    nc.tensor.matmul(pcs[mb][:], ones_row[:],
                                     bias_row[:, q * 512:(q + 1) * 512],
                                     start=False, stop=True)
                    osb = osbp.tile([128, 512], F32, tag="osb",
                                    name=f"o{q}_{mb}")
                    nc.scalar.copy(osb[:], pcs[mb][:])
                    nc.scalar.dma_start(
                        out=out_d[mb * 128:(mb + 1) * 128,
                                  q * 512:(q + 1) * 512],
                        in_=osb[:])

            with nc.named_scope("t_h0"):
                transform_half(0)
            with nc.named_scope("q0"):
                quarter_sweep(0)
            with nc.named_scope("t_h1"):
                transform_half(1)
            with nc.named_scope("q1"):
                quarter_sweep(1)
            with nc.named_scope("q2"):
                quarter_sweep(2)
            with nc.named_scope("q3"):
                quarter_sweep(3)
    nc.finalize()
    return nc


def _prep_inputs(x, weight, bias):
    wtT = np.ascontiguousarray(weight.T)          # [K, N]
    wmin = float(wtT.min())
    wmax = float(wtT.max())
    s = (G_MAX - G_MIN) / (wmax - wmin)
    a = G_MIN / s - wmin
    kappa = wmin - G_MIN / s
    p_idx = np.arange(128, dtype=np.float64)
    mmx = np.zeros((128, 4), dtype=np.float32)
    mmx[:, 0] = a
    mmx[:, 1] = kappa
    # rp[p, kt] = -2*s*(128*kt + p)  (so c = c0b + rp)
    rp = np.zeros((128, KT), dtype=np.float16)
    for kt in range(KT):
        rp[:, kt] = (-2.0 * s * (128.0 * kt + p_idx)).astype(np.float16)
    # c0b[p, n] = s*(4098 + 2n), same for all partitions
    coln = np.arange(N, dtype=np.float64)[None, :]
    c0 = np.ascontiguousarray(np.broadcast_to(
        (np.float64(s) * (4098.0 + 2.0 * coln)).astype(np.float16), (128, N)))

    # wt blocked [h, kt] -> [128, 1024], fp16
    wt_b = np.ascontiguousarray(
        wtT.reshape(KT, 128, NH, 1024).transpose(2, 0, 1, 3)
        .reshape(NH * KT * 128, 1024).astype(np.float16))

    bias2 = np.ascontiguousarray(bias.reshape(1, N)).astype(np.float16)
    in_maps = []
    for c in range(N_CORES):
        x_c = x[c * BC:(c + 1) * BC, :]           # [BC, K]
        # xh[p, kt, m] = x_c[m, kt*128+p]
        xh = np.ascontiguousarray(
            x_c.T.reshape(KT, 128, BC).transpose(1, 0, 2)
            .reshape(128, KT * BC).astype(np.float16))
        in_maps.append({"wt": wt_b, "xt": xh, "bias": bias2, "mmx": mmx,
                        "rp": rp, "c0": c0})
    return in_maps


def _run(x, weight, bias, trace=False, trace_kwargs=None):
    if "nc" not in _CACHE:
        _CACHE["nc"] = _build_nc()
    nc = _CACHE["nc"]
    in_maps = _prep_inputs(x, weight, bias)
    res = run_bass_kernel_spmd(nc, in_maps, list(range(N_CORES)), trace=trace,
                               **(trace_kwargs or {}))
    out = np.concatenate([res.results[c]["out"] for c in range(N_CORES)], axis=0)
    return out, res


def kernel(x, weight, bias):
    x = np.asarray(x, dtype=np.float32)
    weight = np.asarray(weight, dtype=np.float32)
    bias = np.asarray(bias, dtype=np.float32)
    out, _ = _run(x, weight, bias, trace=False)
    return out.astype(np.float32)
